# revision 26
# baseline (speedup 1.0000x reference)
"""Trainium2 Bass kernel for nn_APIHyperInputLayer (hypernet MLP, 8-core data parallel).

Math (per branch):
    h   = prelu(F @ W1 + b1, alpha)                       [R, 64]
    w   = (h @ W2 + b2).reshape(R, F, 128)
    hid = einsum('rf,rfo->ro', F, w)
    out = hid.reshape(E, n, 128).sum(1)                   [E, 128]

Key restructuring: pull the agent-sum inside the W2 contraction.
    S[k, e, f]  = sum_i h[(e,i), k] * F[(e,i), f]     (outer-product episode sums)
    out[e, o]   = sum_{k,f} S[k, e, f] * W2[k, f*128+o]  (+ bias via Fsum row)
This cuts FLOPs ~8.7x vs materializing w.

On-chip schedule per core (256 episodes), all matmuls bf16 -> fp32 PSUM:
  A: h_aug[row, 0:65] = max(x, alpha*x) of F@[W1|b1-augmented]; col 64 == 1.
     (alpha is folded into a doubled W1; PReLU becomes one vector max)
  B: per group of 12 (ally) / 11 (enemy) episodes: S' = h_aug.T @ M where M is the
     block-diagonal masked feature tensor, built by one diagonal-AP DMA into
     pre-zeroed SBUF. S'[64] = per-episode feature sums (h_aug ones column).
  C: out_T[o, e] accumulated over 48+32 f-slices:
     out_T += W2aug[:, f*128:(f+1)*128].T @ S'[:, f::48]   (bias row folded).
Output per core: [128 o, 256 e] fp32; host transposes/concats.
"""

import os
import sys
import functools

import numpy as np

for _p in ("/opt/trn_rl_repo", os.path.expanduser("~/.axon_site/_ro/trn_rl_repo")):
    if os.path.isdir(_p) and _p not in sys.path:
        sys.path.insert(0, _p)

import dataclasses

import ml_dtypes

import concourse.bass as bass
import concourse.bacc as bacc
import concourse.mybir as mybir
import concourse.tile as tile
from concourse.bass_utils import run_bass_kernel_spmd

BF16 = mybir.dt.bfloat16
F32 = mybir.dt.float32

# Problem constants (hardcoded per contest rules)
N_CORES = 8
N_AGENTS, N_ENEMIES = 10, 11
ALLY_F, ENEMY_F = 48, 32
HYPER = 64
OUT = 128
B_FULL = 2048
E_C = B_FULL // N_CORES            # episodes per core = 256
RA = E_C * N_AGENTS                # ally rows per core = 2560
RE = E_C * N_ENEMIES               # enemy rows per core = 2816

# group sizes (episodes per stage-B matmul group; rows <= 128,
# group width * 4B <= 2KB so one stage-B matmul fits one PSUM bank)
EG_A = 10                          # 100 rows, width 480
EG_E = 11                          # 121 rows, width 352

H1 = HYPER + 1                     # 65: h columns + ones col
W1COLS = H1                        # 65: W1 cols + ones column

# wpack column layout (bf16, 65 partitions)
W2A_OFF = 0
W2A_LEN = ALLY_F * OUT             # 6144
W2E_OFF = W2A_OFF + W2A_LEN
W2E_LEN = ENEMY_F * OUT            # 4096
W1A_OFF = W2E_OFF + W2E_LEN
W1E_OFF = W1A_OFF + W1COLS
WPACK_COLS = W1E_OFF + W1COLS


def _groups(n_ep, eg):
    """List of (episode_start, n_episodes) per group."""
    out = []
    e = 0
    while e < n_ep:
        g = min(eg, n_ep - e)
        out.append((e, g))
        e += g
    return out


GROUPS_A = _groups(E_C, EG_A)      # 25 x 10 + 1 x 6
GROUPS_E = _groups(E_C, EG_E)      # 23 x 11 + 1 x 3
MA_FREE = len(GROUPS_A) * EG_A * ALLY_F     # M_all ally free size
ME_FREE = len(GROUPS_E) * EG_E * ENEMY_F
SA_FREE = E_C * ALLY_F             # 12288
SE_FREE = E_C * ENEMY_F            # 8192
HA_FREE = len(GROUPS_A) * H1
HE_FREE = len(GROUPS_E) * H1


def _ap(t, offset, dims):
    """Custom flat AP: dims = [(step, num), ...]; t is an AP or tensor handle."""
    a = t if isinstance(t, bass.AP) else t.ap()
    return dataclasses.replace(a, offset=offset, ap=[[s, n] for (s, n) in dims])


def build_program(alpha_a=0.25, alpha_e=0.25):
    nc = bacc.Bacc("TRN2", target_bir_lowering=False, debug=False)

    # DRAM parameters (per-core shards; bf16 except output)
    fa = nc.declare_dram_parameter("fa", [RA, ALLY_F], BF16, isOutput=False)
    fe = nc.declare_dram_parameter("fe", [RE, ENEMY_F], BF16, isOutput=False)
    fta = nc.declare_dram_parameter("fta", [ALLY_F + 1, RA], BF16, isOutput=False)
    fte = nc.declare_dram_parameter("fte", [ENEMY_F + 1, RE], BF16, isOutput=False)
    wpack = nc.declare_dram_parameter("wpack", [H1, WPACK_COLS], BF16, isOutput=False)
    out_d = nc.declare_dram_parameter("out", [OUT, E_C], F32, isOutput=True)

    with tile.TileContext(nc) as tc:
        _emit(nc, tc, fa, fe, fta, fte, wpack, out_d, alpha_a, alpha_e)
    nc.compile()
    return nc


def _emit(nc, tc, fa, fe, fta, fte, wpack, out_d, alpha_a=0.25, alpha_e=0.25):
    from contextlib import ExitStack

    ctx = ExitStack()
    with ctx:
        const = ctx.enter_context(tc.tile_pool(name="const", bufs=1))
        work = ctx.enter_context(tc.tile_pool(name="work", bufs=1))
        psA = ctx.enter_context(tc.tile_pool(name="psA", bufs=4, space="PSUM"))
        psB = ctx.enter_context(tc.tile_pool(name="psB", bufs=2, space="PSUM"))
        psC = ctx.enter_context(tc.tile_pool(name="psC", bufs=1, space="PSUM"))
        upool = ctx.enter_context(tc.tile_pool(name="upool", bufs=3))

        # ---- persistent SBUF buffers ----
        wp_sb = const.tile([H1, WPACK_COLS], BF16)
        fta_sb = const.tile([ALLY_F + 1, RA], BF16)
        fte_sb = const.tile([ENEMY_F + 1, RE], BF16)
        ma_sb = work.tile([128, MA_FREE], BF16)
        me_sb = work.tile([128, ME_FREE], BF16)
        ha_sb = work.tile([128, HA_FREE], BF16)
        he_sb = work.tile([128, HE_FREE], BF16)
        sa_sb = work.tile([H1, SA_FREE], BF16)
        se_sb = work.tile([H1, SE_FREE], BF16)
        out_sb = work.tile([OUT, E_C], F32)

        # ---- loads ----
        nc.gpsimd.dma_start(wp_sb[:], wpack.ap())
        nc.gpsimd.dma_start(fta_sb[:], fta.ap())
        nc.gpsimd.dma_start(fte_sb[:], fte.ap())

        # zero the masked-feature buffers (ally first: its diag DMAs wait
        # DVE>=1 on fresh lanes; enemy ring-first DMAs get lanes 6,7 kept
        # fresh below so their DVE>=2 wait is their only one).
        nc.vector.memset(ma_sb[:], 0.0)
        nc.vector.memset(me_sb[:], 0.0)

        # ---- diagonal DMAs: DRAM features -> block-diagonal M ----
        # One DMA per episode-slot e_local: for fixed e_local the SBUF
        # destination has pure strides (group dim steps free only, agent
        # dim steps whole partitions) so the BIR verifier accepts it.
        # HWDGE (sync/scalar) DMA instrs carry at most ONE sync wait;
        # SWDGE (gpsimd) waits are software -> flexible. Route the
        # dep-heavy first DMAs through gpsimd.
        dma_engines = [nc.scalar, nc.sync]
        dma_rr = [0]

        def diag_dma(m_sb, f_d, groups, eg, n_per, featf, mfree, swdge_els=()):
            gstride = eg * featf
            tail_g = groups[-1][1]          # episodes in last (ragged) group
            nfull = len(groups) - (1 if tail_g != eg else 0)
            for el in range(eg):
                ng = nfull + (1 if el < tail_g and tail_g != eg else 0)
                if el in swdge_els:
                    eng = nc.gpsimd
                else:
                    eng = dma_engines[dma_rr[0] % len(dma_engines)]
                    dma_rr[0] += 1
                eng.dma_start(
                    _ap(m_sb, (el * n_per) * mfree + el * featf, [
                        (mfree, n_per),         # agent: whole partitions
                        (gstride, ng),          # group: free step only
                        (1, featf),
                    ]),
                    _ap(f_d, el * n_per * featf, [
                        (featf, n_per),
                        (eg * n_per * featf, ng),
                        (1, featf),
                    ]),
                )

        diag_dma(ma_sb, fa, GROUPS_A, EG_A, N_AGENTS, ALLY_F, MA_FREE,
                 swdge_els=(6, 7, 8, 9))
        diag_dma(me_sb, fe, GROUPS_E, EG_E, N_ENEMIES, ENEMY_F, ME_FREE)


        # ---- stage A (layer 1 + PReLU) for both branches ----
        def stage_a(groups, n_per, featf, ft_sb, w1_off, h_sb, alpha):
            fp1 = featf + 1
            for gi, (e0, g) in enumerate(groups):
                rows = g * n_per
                r0 = e0 * n_per
                pa = psA.tile([128, W1COLS], F32, tag="psA")
                nc.tensor.matmul(
                    pa[0:rows, :],
                    ft_sb[0:fp1, r0:r0 + rows],
                    wp_sb[0:fp1, w1_off:w1_off + W1COLS],
                    start=True, stop=True,
                )
                # PReLU (0<=alpha<=1): u = alpha*x; h = max(x, u).
                # ones col: max(1, alpha) == 1. Only one PSUM input per op.
                ut = upool.tile([128, H1], BF16, tag="u")
                nc.vector.tensor_scalar_mul(
                    ut[0:rows, :], pa[0:rows, 0:H1], alpha)
                nc.vector.tensor_max(
                    h_sb[0:rows, gi * H1:(gi + 1) * H1],
                    pa[0:rows, 0:H1],
                    ut[0:rows, :],
                )

        stage_a(GROUPS_A, N_AGENTS, ALLY_F, fta_sb, W1A_OFF, ha_sb, alpha_a)
        stage_a(GROUPS_E, N_ENEMIES, ENEMY_F, fte_sb, W1E_OFF, he_sb, alpha_e)

        # ---- stage B (episode outer-product sums) ----
        def stage_b(groups, eg, n_per, featf, m_sb, h_sb, s_sb):
            gstride = eg * featf
            for gi, (e0, g) in enumerate(groups):
                rows = g * n_per
                width = g * featf
                moff = gi * gstride
                pb = psB.tile([H1, 512], F32, tag="psB")
                nc.tensor.matmul(
                    pb[:, 0:width],
                    h_sb[0:rows, gi * H1:(gi + 1) * H1],
                    m_sb[0:rows, moff:moff + width],
                    start=True, stop=True,
                )
                nc.vector.tensor_copy(
                    s_sb[:, e0 * featf:e0 * featf + width],
                    pb[:, 0:width],
                )

        stage_b(GROUPS_A, EG_A, N_AGENTS, ALLY_F, ma_sb, ha_sb, sa_sb)
        stage_b(GROUPS_E, EG_E, N_ENEMIES, ENEMY_F, me_sb, he_sb, se_sb)

        # ---- stage C: out_T[o, e] accumulation over 80 f-slices ----
        pc = psC.tile([OUT, E_C], F32)
        n_slices = ALLY_F + ENEMY_F
        idx = 0
        for f in range(ALLY_F):
            nc.tensor.matmul(
                pc[:],
                wp_sb[:, W2A_OFF + f * OUT:W2A_OFF + (f + 1) * OUT],
                _ap(sa_sb, f, [(SA_FREE, H1), (ALLY_F, E_C)]),
                start=(idx == 0), stop=(idx == n_slices - 1),
            )
            idx += 1
        for f in range(ENEMY_F):
            nc.tensor.matmul(
                pc[:],
                wp_sb[:, W2E_OFF + f * OUT:W2E_OFF + (f + 1) * OUT],
                _ap(se_sb, f, [(SE_FREE, H1), (ENEMY_F, E_C)]),
                start=(idx == 0), stop=(idx == n_slices - 1),
            )
            idx += 1

        nc.vector.tensor_copy(out_sb[:], pc[:])
        nc.gpsimd.dma_start(out_d.ap(), out_sb[:])


@functools.lru_cache(maxsize=2)
def _cached_program(alpha_a, alpha_e):
    return build_program(alpha_a, alpha_e)


def host_prep(ally_features, enemy_features, Wa1, ba1, aa, Wa2, ba2,
              We1, be1, ae, We2, be2):
    """Build per-core input maps (numpy, bf16)."""
    bf = ml_dtypes.bfloat16

    def w1_pack(W1, b1, featf):
        w = np.zeros((H1, W1COLS), dtype=np.float32)
        w[0:featf, 0:HYPER] = np.asarray(W1)
        w[featf, 0:HYPER] = np.asarray(b1)
        w[featf, HYPER] = 1.0                 # ones column
        return w

    def uniform_alpha(a):
        a = np.asarray(a, dtype=np.float32)
        assert np.allclose(a, a[0]), "per-channel alpha not supported"
        assert 0.0 <= float(a[0]) <= 1.0, "alpha outside [0,1]"
        return float(a[0])

    ua, ue = uniform_alpha(aa), uniform_alpha(ae)
    w1a = w1_pack(Wa1, ba1, ALLY_F)
    w1e = w1_pack(We1, be1, ENEMY_F)

    wp = np.zeros((H1, WPACK_COLS), dtype=np.float32)
    wp[0:HYPER, W2A_OFF:W2A_OFF + W2A_LEN] = np.asarray(Wa2)
    wp[HYPER, W2A_OFF:W2A_OFF + W2A_LEN] = np.asarray(ba2)
    wp[0:HYPER, W2E_OFF:W2E_OFF + W2E_LEN] = np.asarray(We2)
    wp[HYPER, W2E_OFF:W2E_OFF + W2E_LEN] = np.asarray(be2)
    wp[:, W1A_OFF:W1A_OFF + W1COLS] = w1a
    wp[:, W1E_OFF:W1E_OFF + W1COLS] = w1e
    wp = wp.astype(bf)

    fa_all = np.asarray(ally_features, dtype=np.float32).astype(bf)
    fe_all = np.asarray(enemy_features, dtype=np.float32).astype(bf)

    in_maps = []
    for c in range(N_CORES):
        fa_c = np.ascontiguousarray(fa_all[c * RA:(c + 1) * RA])
        fe_c = np.ascontiguousarray(fe_all[c * RE:(c + 1) * RE])
        fta_c = np.concatenate(
            [fa_c.T.astype(np.float32),
             np.ones((1, RA), dtype=np.float32)], axis=0).astype(bf)
        fte_c = np.concatenate(
            [fe_c.T.astype(np.float32),
             np.ones((1, RE), dtype=np.float32)], axis=0).astype(bf)
        in_maps.append({
            "fa": fa_c, "fe": fe_c,
            "fta": np.ascontiguousarray(fta_c),
            "fte": np.ascontiguousarray(fte_c),
            "wpack": wp,
        })
    return in_maps, ua, ue


def kernel(**inputs) -> np.ndarray:
    in_maps, ua, ue = host_prep(**inputs)
    nc = _cached_program(ua, ue)
    res = run_bass_kernel_spmd(nc, in_maps, core_ids=list(range(N_CORES)))
    outs = [np.asarray(r["out"], dtype=np.float32) for r in res.results]
    return np.concatenate([o.T for o in outs], axis=0)


if __name__ == "__main__":
    build_program()
    print("built ok")


# revision 27
# speedup vs baseline: 1.1854x; 1.1854x over previous
"""Trainium2 Bass kernel for nn_APIHyperInputLayer (hypernet MLP, 8-core data parallel).

Math (per branch):
    h   = prelu(F @ W1 + b1, alpha)                       [R, 64]
    w   = (h @ W2 + b2).reshape(R, F, 128)
    hid = einsum('rf,rfo->ro', F, w)
    out = hid.reshape(E, n, 128).sum(1)                   [E, 128]

Key restructuring: pull the agent-sum inside the W2 contraction.
    S[k, e, f]  = sum_i h[(e,i), k] * F[(e,i), f]     (outer-product episode sums)
    out[e, o]   = sum_{k,f} S[k, e, f] * W2[k, f*128+o]  (+ bias via Fsum row)
This cuts FLOPs ~8.7x vs materializing w.

On-chip schedule per core (256 episodes), all matmuls bf16 -> fp32 PSUM:
  A: h_aug[row, 0:65] = max(x, alpha*x) of F@[W1|b1-augmented]; col 64 == 1.
     (alpha is folded into a doubled W1; PReLU becomes one vector max)
  B: per group of 12 (ally) / 11 (enemy) episodes: S' = h_aug.T @ M where M is the
     block-diagonal masked feature tensor, built by one diagonal-AP DMA into
     pre-zeroed SBUF. S'[64] = per-episode feature sums (h_aug ones column).
  C: out_T[o, e] accumulated over 48+32 f-slices:
     out_T += W2aug[:, f*128:(f+1)*128].T @ S'[:, f::48]   (bias row folded).
Output per core: [128 o, 256 e] fp32; host transposes/concats.
"""

import os
import sys
import functools

import numpy as np

for _p in ("/opt/trn_rl_repo", os.path.expanduser("~/.axon_site/_ro/trn_rl_repo")):
    if os.path.isdir(_p) and _p not in sys.path:
        sys.path.insert(0, _p)

import dataclasses

import ml_dtypes

import concourse.bass as bass
import concourse.bacc as bacc
import concourse.mybir as mybir
import concourse.tile as tile
from concourse.bass_utils import run_bass_kernel_spmd

BF16 = mybir.dt.bfloat16
F32 = mybir.dt.float32

# Problem constants (hardcoded per contest rules)
N_CORES = 8
N_AGENTS, N_ENEMIES = 10, 11
ALLY_F, ENEMY_F = 48, 32
HYPER = 64
OUT = 128
B_FULL = 2048
E_C = B_FULL // N_CORES            # episodes per core = 256
RA = E_C * N_AGENTS                # ally rows per core = 2560
RE = E_C * N_ENEMIES               # enemy rows per core = 2816

# group sizes (episodes per stage-B matmul group; rows <= 128,
# group width * 4B <= 2KB so one stage-B matmul fits one PSUM bank)
EG_A = 10                          # 100 rows, width 480
EG_E = 11                          # 121 rows, width 352

H1 = HYPER + 1                     # 65: h columns + ones col
W1COLS = H1                        # 65: W1 cols + ones column

# wpack column layout (bf16, 65 partitions)
W2A_OFF = 0
W2A_LEN = ALLY_F * OUT             # 6144
W2E_OFF = W2A_OFF + W2A_LEN
W2E_LEN = ENEMY_F * OUT            # 4096
W1A_OFF = W2E_OFF + W2E_LEN
W1E_OFF = W1A_OFF + W1COLS
WPACK_COLS = W1E_OFF + W1COLS


def _groups(n_ep, eg):
    """List of (episode_start, n_episodes) per group."""
    out = []
    e = 0
    while e < n_ep:
        g = min(eg, n_ep - e)
        out.append((e, g))
        e += g
    return out


GROUPS_A = _groups(E_C, EG_A)      # 25 x 10 + 1 x 6
GROUPS_E = _groups(E_C, EG_E)      # 23 x 11 + 1 x 3
MA_FREE = len(GROUPS_A) * EG_A * ALLY_F     # M_all ally free size
ME_FREE = len(GROUPS_E) * EG_E * ENEMY_F
SA_FREE = E_C * ALLY_F             # 12288
SE_FREE = E_C * ENEMY_F            # 8192
HA_FREE = len(GROUPS_A) * H1
HE_FREE = len(GROUPS_E) * H1


def _ap(t, offset, dims):
    """Custom flat AP: dims = [(step, num), ...]; t is an AP or tensor handle."""
    a = t if isinstance(t, bass.AP) else t.ap()
    return dataclasses.replace(a, offset=offset, ap=[[s, n] for (s, n) in dims])


def build_program(alpha_a=0.25, alpha_e=0.25):
    nc = bacc.Bacc("TRN2", target_bir_lowering=False, debug=False)

    # DRAM parameters (per-core shards; bf16 except output)
    fa = nc.declare_dram_parameter("fa", [RA, ALLY_F], BF16, isOutput=False)
    fe = nc.declare_dram_parameter("fe", [RE, ENEMY_F], BF16, isOutput=False)
    fta = nc.declare_dram_parameter("fta", [ALLY_F + 1, RA], BF16, isOutput=False)
    fte = nc.declare_dram_parameter("fte", [ENEMY_F + 1, RE], BF16, isOutput=False)
    wpack = nc.declare_dram_parameter("wpack", [H1, WPACK_COLS], BF16, isOutput=False)
    out_d = nc.declare_dram_parameter("out", [OUT, E_C], F32, isOutput=True)

    with tile.TileContext(nc) as tc:
        _emit(nc, tc, fa, fe, fta, fte, wpack, out_d, alpha_a, alpha_e)
    nc.compile()
    return nc


def _emit(nc, tc, fa, fe, fta, fte, wpack, out_d, alpha_a=0.25, alpha_e=0.25):
    from contextlib import ExitStack

    ctx = ExitStack()
    with ctx:
        const = ctx.enter_context(tc.tile_pool(name="const", bufs=1))
        work = ctx.enter_context(tc.tile_pool(name="work", bufs=1))
        psA = ctx.enter_context(tc.tile_pool(name="psA", bufs=3, space="PSUM"))
        psB = ctx.enter_context(tc.tile_pool(name="psB", bufs=4, space="PSUM"))
        psC = ctx.enter_context(tc.tile_pool(name="psC", bufs=1, space="PSUM"))
        upool = ctx.enter_context(tc.tile_pool(name="upool", bufs=3))

        # ---- persistent SBUF buffers ----
        wp_sb = const.tile([H1, WPACK_COLS], BF16)
        fta_sb = const.tile([ALLY_F + 1, RA], BF16)
        fte_sb = const.tile([ENEMY_F + 1, RE], BF16)
        ma_sb = work.tile([128, MA_FREE], BF16)
        me_sb = work.tile([128, ME_FREE], BF16)
        ha_sb = work.tile([128, HA_FREE], BF16)
        he_sb = work.tile([128, HE_FREE], BF16)
        sa_sb = work.tile([H1, SA_FREE], BF16)
        se_sb = work.tile([H1, SE_FREE], BF16)
        out_sb = work.tile([OUT, E_C], F32)

        # ---- loads ----
        nc.sync.dma_start(wp_sb[:], wpack.ap())
        nc.sync.dma_start(fta_sb[:], fta.ap())
        nc.scalar.dma_start(fte_sb[:], fte.ap())

        # zero the masked-feature buffers (ally first: its diag DMAs wait
        # DVE>=1 on fresh lanes; enemy ring-first DMAs get lanes 6,7 kept
        # fresh below so their DVE>=2 wait is their only one).
        nc.vector.memset(ma_sb[:].bitcast(F32), 0.0)
        nc.vector.memset(me_sb[:].bitcast(F32), 0.0)

        # ---- diagonal DMAs: DRAM features -> block-diagonal M ----
        # One DMA per episode-slot e_local: for fixed e_local the SBUF
        # destination has pure strides (group dim steps free only, agent
        # dim steps whole partitions) so the BIR verifier accepts it.
        # HWDGE (sync/scalar) DMA instrs carry at most ONE sync wait;
        # SWDGE (gpsimd) waits are software -> flexible. Route the
        # dep-heavy first DMAs through gpsimd.
        dma_engines = [nc.scalar, nc.sync]
        dma_rr = [0]

        def diag_dma(m_sb, f_d, groups, eg, n_per, featf, mfree, swdge_els=()):
            gstride = eg * featf
            tail_g = groups[-1][1]          # episodes in last (ragged) group
            nfull = len(groups) - (1 if tail_g != eg else 0)
            for el in range(eg):
                ng = nfull + (1 if el < tail_g and tail_g != eg else 0)
                if el in swdge_els:
                    eng = nc.gpsimd
                else:
                    eng = dma_engines[dma_rr[0] % len(dma_engines)]
                    dma_rr[0] += 1
                eng.dma_start(
                    _ap(m_sb, (el * n_per) * mfree + el * featf, [
                        (mfree, n_per),         # agent: whole partitions
                        (gstride, ng),          # group: free step only
                        (1, featf),
                    ]),
                    _ap(f_d, el * n_per * featf, [
                        (featf, n_per),
                        (eg * n_per * featf, ng),
                        (1, featf),
                    ]),
                )

        diag_dma(ma_sb, fa, GROUPS_A, EG_A, N_AGENTS, ALLY_F, MA_FREE)
        diag_dma(me_sb, fe, GROUPS_E, EG_E, N_ENEMIES, ENEMY_F, ME_FREE)


        # ---- stage A (layer 1 + PReLU) for both branches ----
        # Two groups share one PSUM tile; PReLU (u = alpha*x; h = max(x,u))
        # is done once per pair to halve the DVE op count.
        def stage_a(groups, n_per, featf, ft_sb, w1_off, h_sb, alpha):
            fp1 = featf + 1
            pairs = [groups[i:i + 2] for i in range(0, len(groups), 2)]
            for pi, pair in enumerate(pairs):
                pa = psA.tile([128, 2 * W1COLS], F32, tag="psA")
                rows_l = []
                for slot, (e0, g) in enumerate(pair):
                    rows = g * n_per
                    rows_l.append(rows)
                    r0 = e0 * n_per
                    nc.tensor.matmul(
                        pa[0:rows, slot * W1COLS:(slot + 1) * W1COLS],
                        ft_sb[0:fp1, r0:r0 + rows],
                        wp_sb[0:fp1, w1_off:w1_off + W1COLS],
                        start=True, stop=True,
                    )
                gi0 = 2 * pi
                if len(pair) == 2 and rows_l[0] == rows_l[1]:
                    rows = rows_l[0]
                    ut = upool.tile([128, 2 * H1], BF16, tag="u")
                    src_ap = _ap(pa, 0, [(2 * W1COLS, rows), (W1COLS, 2), (1, H1)])
                    nc.vector.tensor_scalar_mul(ut[0:rows, :], src_ap, alpha)
                    nc.vector.tensor_max(
                        h_sb[0:rows, gi0 * H1:(gi0 + 2) * H1],
                        src_ap, ut[0:rows, :])
                else:
                    for slot in range(len(pair)):
                        rows = rows_l[slot]
                        ut = upool.tile([128, 2 * H1], BF16, tag="u")
                        nc.vector.tensor_scalar_mul(
                            ut[0:rows, 0:H1],
                            pa[0:rows, slot * W1COLS:slot * W1COLS + H1], alpha)
                        nc.vector.tensor_max(
                            h_sb[0:rows, (gi0 + slot) * H1:(gi0 + slot + 1) * H1],
                            pa[0:rows, slot * W1COLS:slot * W1COLS + H1],
                            ut[0:rows, 0:H1])

        stage_a(GROUPS_A, N_AGENTS, ALLY_F, fta_sb, W1A_OFF, ha_sb, alpha_a)
        stage_a(GROUPS_E, N_ENEMIES, ENEMY_F, fte_sb, W1E_OFF, he_sb, alpha_e)

        # ---- stage B (episode outer-product sums) ----
        def stage_b(groups, eg, n_per, featf, m_sb, h_sb, s_sb):
            gstride = eg * featf
            for gi, (e0, g) in enumerate(groups):
                rows = g * n_per
                width = g * featf
                moff = gi * gstride
                pb = psB.tile([H1, 512], F32, tag="psB")
                nc.tensor.matmul(
                    pb[:, 0:width],
                    h_sb[0:rows, gi * H1:(gi + 1) * H1],
                    m_sb[0:rows, moff:moff + width],
                    start=True, stop=True,
                )
                if gi % 2 == 0:
                    nc.vector.tensor_copy(
                        s_sb[:, e0 * featf:e0 * featf + width],
                        pb[:, 0:width],
                    )
                else:
                    nc.scalar.copy(
                        s_sb[:, e0 * featf:e0 * featf + width],
                        pb[:, 0:width],
                    )

        stage_b(GROUPS_A, EG_A, N_AGENTS, ALLY_F, ma_sb, ha_sb, sa_sb)
        stage_b(GROUPS_E, EG_E, N_ENEMIES, ENEMY_F, me_sb, he_sb, se_sb)

        # ---- stage C: out_T[o, e] accumulation over 80 f-slices ----
        pc = psC.tile([OUT, E_C], F32)
        n_slices = ALLY_F + ENEMY_F
        idx = 0
        for f in range(ALLY_F):
            nc.tensor.matmul(
                pc[:],
                wp_sb[:, W2A_OFF + f * OUT:W2A_OFF + (f + 1) * OUT],
                _ap(sa_sb, f, [(SA_FREE, H1), (ALLY_F, E_C)]),
                start=(idx == 0), stop=(idx == n_slices - 1),
            )
            idx += 1
        for f in range(ENEMY_F):
            nc.tensor.matmul(
                pc[:],
                wp_sb[:, W2E_OFF + f * OUT:W2E_OFF + (f + 1) * OUT],
                _ap(se_sb, f, [(SE_FREE, H1), (ENEMY_F, E_C)]),
                start=(idx == 0), stop=(idx == n_slices - 1),
            )
            idx += 1

        nc.vector.tensor_copy(out_sb[:], pc[:])
        nc.sync.dma_start(out_d.ap(), out_sb[:])


@functools.lru_cache(maxsize=2)
def _cached_program(alpha_a, alpha_e):
    return build_program(alpha_a, alpha_e)


def host_prep(ally_features, enemy_features, Wa1, ba1, aa, Wa2, ba2,
              We1, be1, ae, We2, be2):
    """Build per-core input maps (numpy, bf16)."""
    bf = ml_dtypes.bfloat16

    def w1_pack(W1, b1, featf):
        w = np.zeros((H1, W1COLS), dtype=np.float32)
        w[0:featf, 0:HYPER] = np.asarray(W1)
        w[featf, 0:HYPER] = np.asarray(b1)
        w[featf, HYPER] = 1.0                 # ones column
        return w

    def uniform_alpha(a):
        a = np.asarray(a, dtype=np.float32)
        assert np.allclose(a, a[0]), "per-channel alpha not supported"
        assert 0.0 <= float(a[0]) <= 1.0, "alpha outside [0,1]"
        return float(a[0])

    ua, ue = uniform_alpha(aa), uniform_alpha(ae)
    w1a = w1_pack(Wa1, ba1, ALLY_F)
    w1e = w1_pack(We1, be1, ENEMY_F)

    wp = np.zeros((H1, WPACK_COLS), dtype=np.float32)
    wp[0:HYPER, W2A_OFF:W2A_OFF + W2A_LEN] = np.asarray(Wa2)
    wp[HYPER, W2A_OFF:W2A_OFF + W2A_LEN] = np.asarray(ba2)
    wp[0:HYPER, W2E_OFF:W2E_OFF + W2E_LEN] = np.asarray(We2)
    wp[HYPER, W2E_OFF:W2E_OFF + W2E_LEN] = np.asarray(be2)
    wp[:, W1A_OFF:W1A_OFF + W1COLS] = w1a
    wp[:, W1E_OFF:W1E_OFF + W1COLS] = w1e
    wp = wp.astype(bf)

    fa_all = np.asarray(ally_features, dtype=np.float32).astype(bf)
    fe_all = np.asarray(enemy_features, dtype=np.float32).astype(bf)

    in_maps = []
    for c in range(N_CORES):
        fa_c = np.ascontiguousarray(fa_all[c * RA:(c + 1) * RA])
        fe_c = np.ascontiguousarray(fe_all[c * RE:(c + 1) * RE])
        fta_c = np.concatenate(
            [fa_c.T.astype(np.float32),
             np.ones((1, RA), dtype=np.float32)], axis=0).astype(bf)
        fte_c = np.concatenate(
            [fe_c.T.astype(np.float32),
             np.ones((1, RE), dtype=np.float32)], axis=0).astype(bf)
        in_maps.append({
            "fa": fa_c, "fe": fe_c,
            "fta": np.ascontiguousarray(fta_c),
            "fte": np.ascontiguousarray(fte_c),
            "wpack": wp,
        })
    return in_maps, ua, ue


def kernel(**inputs) -> np.ndarray:
    in_maps, ua, ue = host_prep(**inputs)
    nc = _cached_program(ua, ue)
    res = run_bass_kernel_spmd(nc, in_maps, core_ids=list(range(N_CORES)))
    outs = [np.asarray(r["out"], dtype=np.float32) for r in res.results]
    return np.concatenate([o.T for o in outs], axis=0)


if __name__ == "__main__":
    build_program()
    print("built ok")


# revision 28
# speedup vs baseline: 1.1978x; 1.0105x over previous
"""Trainium2 Bass kernel for nn_APIHyperInputLayer (hypernet MLP, 8-core data parallel).

Math (per branch):
    h   = prelu(F @ W1 + b1, alpha)                       [R, 64]
    w   = (h @ W2 + b2).reshape(R, F, 128)
    hid = einsum('rf,rfo->ro', F, w)
    out = hid.reshape(E, n, 128).sum(1)                   [E, 128]

Key restructuring: pull the agent-sum inside the W2 contraction.
    S[k, e, f]  = sum_i h[(e,i), k] * F[(e,i), f]     (outer-product episode sums)
    out[e, o]   = sum_{k,f} S[k, e, f] * W2[k, f*128+o]  (+ bias via Fsum row)
This cuts FLOPs ~8.7x vs materializing w.

On-chip schedule per core (256 episodes), all matmuls bf16 -> fp32 PSUM:
  A: h_aug[row, 0:65] = max(x, alpha*x) of F@[W1|b1-augmented]; col 64 == 1.
     (alpha is folded into a doubled W1; PReLU becomes one vector max)
  B: per group of 12 (ally) / 11 (enemy) episodes: S' = h_aug.T @ M where M is the
     block-diagonal masked feature tensor, built by one diagonal-AP DMA into
     pre-zeroed SBUF. S'[64] = per-episode feature sums (h_aug ones column).
  C: out_T[o, e] accumulated over 48+32 f-slices:
     out_T += W2aug[:, f*128:(f+1)*128].T @ S'[:, f::48]   (bias row folded).
Output per core: [128 o, 256 e] fp32; host transposes/concats.
"""

import os
import sys
import functools

import numpy as np

for _p in ("/opt/trn_rl_repo", os.path.expanduser("~/.axon_site/_ro/trn_rl_repo")):
    if os.path.isdir(_p) and _p not in sys.path:
        sys.path.insert(0, _p)

import dataclasses

import ml_dtypes

import concourse.bass as bass
import concourse.bacc as bacc
import concourse.mybir as mybir
import concourse.tile as tile
from concourse.bass_utils import run_bass_kernel_spmd

BF16 = mybir.dt.bfloat16
F32 = mybir.dt.float32

# Problem constants (hardcoded per contest rules)
N_CORES = 8
N_AGENTS, N_ENEMIES = 10, 11
ALLY_F, ENEMY_F = 48, 32
HYPER = 64
OUT = 128
B_FULL = 2048
E_C = B_FULL // N_CORES            # episodes per core = 256
RA = E_C * N_AGENTS                # ally rows per core = 2560
RE = E_C * N_ENEMIES               # enemy rows per core = 2816

# group sizes (episodes per stage-B matmul group; rows <= 128,
# group width * 4B <= 2KB so one stage-B matmul fits one PSUM bank)
EG_A = 10                          # 100 rows, width 480
EG_E = 11                          # 121 rows, width 352

H1 = HYPER + 1                     # 65: h columns + ones col
W1COLS = H1                        # 65: W1 cols + ones column

# w2pack column layout (bf16, 65 partitions; loaded late, needed by stage C)
W2A_OFF = 0
W2A_LEN = ALLY_F * OUT             # 6144
W2E_OFF = W2A_OFF + W2A_LEN
W2E_LEN = ENEMY_F * OUT            # 4096
W2PACK_COLS = W2E_OFF + W2E_LEN
# w1pack: tiny, loaded first, padded to 128 partitions so every SDMA engine
# participates and the completion semaphore fires promptly
W1A_OFF = 0
W1E_OFF = W1COLS
W1PACK_COLS = 2 * W1COLS


def _groups(n_ep, eg):
    """List of (episode_start, n_episodes) per group."""
    out = []
    e = 0
    while e < n_ep:
        g = min(eg, n_ep - e)
        out.append((e, g))
        e += g
    return out


GROUPS_A = _groups(E_C, EG_A)      # 25 x 10 + 1 x 6
GROUPS_E = _groups(E_C, EG_E)      # 23 x 11 + 1 x 3
MA_FREE = len(GROUPS_A) * EG_A * ALLY_F     # M_all ally free size
ME_FREE = len(GROUPS_E) * EG_E * ENEMY_F
SA_FREE = E_C * ALLY_F             # 12288
SE_FREE = E_C * ENEMY_F            # 8192
HA_FREE = len(GROUPS_A) * H1
HE_FREE = len(GROUPS_E) * H1


def _ap(t, offset, dims):
    """Custom flat AP: dims = [(step, num), ...]; t is an AP or tensor handle."""
    a = t if isinstance(t, bass.AP) else t.ap()
    return dataclasses.replace(a, offset=offset, ap=[[s, n] for (s, n) in dims])


def build_program(alpha_a=0.25, alpha_e=0.25):
    nc = bacc.Bacc("TRN2", target_bir_lowering=False, debug=False)

    # DRAM parameters (per-core shards; bf16 except output)
    fa = nc.declare_dram_parameter("fa", [RA, ALLY_F], BF16, isOutput=False)
    fe = nc.declare_dram_parameter("fe", [RE, ENEMY_F], BF16, isOutput=False)
    fta = nc.declare_dram_parameter("fta", [128, RA], BF16, isOutput=False)
    fte = nc.declare_dram_parameter("fte", [128, RE], BF16, isOutput=False)
    w1pack = nc.declare_dram_parameter("w1pack", [128, W1PACK_COLS], BF16, isOutput=False)
    w2pack = nc.declare_dram_parameter("w2pack", [H1, W2PACK_COLS], BF16, isOutput=False)
    out_d = nc.declare_dram_parameter("out", [OUT, E_C], F32, isOutput=True)

    with tile.TileContext(nc) as tc:
        _emit(nc, tc, fa, fe, fta, fte, w1pack, w2pack, out_d, alpha_a, alpha_e)
    nc.compile()
    return nc


def _emit(nc, tc, fa, fe, fta, fte, w1pack, w2pack, out_d, alpha_a=0.25, alpha_e=0.25):
    from contextlib import ExitStack

    ctx = ExitStack()
    with ctx:
        const = ctx.enter_context(tc.tile_pool(name="const", bufs=1))
        work = ctx.enter_context(tc.tile_pool(name="work", bufs=1))
        psA = ctx.enter_context(tc.tile_pool(name="psA", bufs=3, space="PSUM"))
        psB = ctx.enter_context(tc.tile_pool(name="psB", bufs=4, space="PSUM"))
        psC = ctx.enter_context(tc.tile_pool(name="psC", bufs=1, space="PSUM"))
        upool = ctx.enter_context(tc.tile_pool(name="upool", bufs=3))

        # ---- persistent SBUF buffers ----
        w1_sb = const.tile([128, W1PACK_COLS], BF16)
        w2_sb = const.tile([H1, W2PACK_COLS], BF16)
        fta_sb = const.tile([128, RA], BF16)
        fte_sb = const.tile([128, RE], BF16)
        ma_sb = work.tile([128, MA_FREE], BF16)
        me_sb = work.tile([128, ME_FREE], BF16)
        ha_sb = work.tile([128, HA_FREE], BF16)
        he_sb = work.tile([128, HE_FREE], BF16)
        sa_sb = work.tile([H1, SA_FREE], BF16)
        se_sb = work.tile([H1, SE_FREE], BF16)
        out_sb = work.tile([OUT, E_C], F32)

        # ---- loads ----
        nc.sync.dma_start(w1_sb[:], w1pack.ap())
        nc.sync.dma_start(fta_sb[:], fta.ap())
        nc.scalar.dma_start(fte_sb[:], fte.ap())
        nc.scalar.dma_start(w2_sb[:], w2pack.ap())

        # zero the masked-feature buffers (ally first: its diag DMAs wait
        # DVE>=1 on fresh lanes; enemy ring-first DMAs get lanes 6,7 kept
        # fresh below so their DVE>=2 wait is their only one).
        nc.vector.memset(ma_sb[:].bitcast(F32), 0.0)
        nc.gpsimd.memset(me_sb[:].bitcast(F32), 0.0)

        # ---- diagonal DMAs: DRAM features -> block-diagonal M ----
        # One DMA per episode-slot e_local: for fixed e_local the SBUF
        # destination has pure strides (group dim steps free only, agent
        # dim steps whole partitions) so the BIR verifier accepts it.
        # HWDGE (sync/scalar) DMA instrs carry at most ONE sync wait;
        # SWDGE (gpsimd) waits are software -> flexible. Route the
        # dep-heavy first DMAs through gpsimd.
        dma_engines = [nc.scalar, nc.sync]
        dma_rr = [0]

        def diag_dma(m_sb, f_d, groups, eg, n_per, featf, mfree, swdge_els=()):
            gstride = eg * featf
            tail_g = groups[-1][1]          # episodes in last (ragged) group
            nfull = len(groups) - (1 if tail_g != eg else 0)
            for el in range(eg):
                ng = nfull + (1 if el < tail_g and tail_g != eg else 0)
                if el in swdge_els:
                    eng = nc.gpsimd
                else:
                    eng = dma_engines[dma_rr[0] % len(dma_engines)]
                    dma_rr[0] += 1
                eng.dma_start(
                    _ap(m_sb, (el * n_per) * mfree + el * featf, [
                        (mfree, n_per),         # agent: whole partitions
                        (gstride, ng),          # group: free step only
                        (1, featf),
                    ]),
                    _ap(f_d, el * n_per * featf, [
                        (featf, n_per),
                        (eg * n_per * featf, ng),
                        (1, featf),
                    ]),
                )

        diag_dma(ma_sb, fa, GROUPS_A, EG_A, N_AGENTS, ALLY_F, MA_FREE)
        diag_dma(me_sb, fe, GROUPS_E, EG_E, N_ENEMIES, ENEMY_F, ME_FREE)


        # ---- stage A (layer 1 + PReLU) for both branches ----
        # Two groups share one PSUM tile; PReLU (u = alpha*x; h = max(x,u))
        # is done once per pair to halve the DVE op count.
        def stage_a(groups, n_per, featf, ft_sb, w1_off, h_sb, alpha):
            fp1 = featf + 1
            pairs = [groups[i:i + 2] for i in range(0, len(groups), 2)]
            for pi, pair in enumerate(pairs):
                pa = psA.tile([128, 2 * W1COLS], F32, tag="psA")
                rows_l = []
                for slot, (e0, g) in enumerate(pair):
                    rows = g * n_per
                    rows_l.append(rows)
                    r0 = e0 * n_per
                    nc.tensor.matmul(
                        pa[0:rows, slot * W1COLS:(slot + 1) * W1COLS],
                        ft_sb[0:fp1, r0:r0 + rows],
                        w1_sb[0:fp1, w1_off:w1_off + W1COLS],
                        start=True, stop=True,
                    )
                gi0 = 2 * pi
                if len(pair) == 2 and rows_l[0] == rows_l[1]:
                    rows = rows_l[0]
                    ut = upool.tile([128, 2 * H1], BF16, tag="u")
                    src_ap = _ap(pa, 0, [(2 * W1COLS, rows), (W1COLS, 2), (1, H1)])
                    nc.vector.tensor_scalar_mul(ut[0:rows, :], src_ap, alpha)
                    nc.vector.tensor_max(
                        h_sb[0:rows, gi0 * H1:(gi0 + 2) * H1],
                        src_ap, ut[0:rows, :])
                else:
                    for slot in range(len(pair)):
                        rows = rows_l[slot]
                        ut = upool.tile([128, 2 * H1], BF16, tag="u")
                        nc.vector.tensor_scalar_mul(
                            ut[0:rows, 0:H1],
                            pa[0:rows, slot * W1COLS:slot * W1COLS + H1], alpha)
                        nc.vector.tensor_max(
                            h_sb[0:rows, (gi0 + slot) * H1:(gi0 + slot + 1) * H1],
                            pa[0:rows, slot * W1COLS:slot * W1COLS + H1],
                            ut[0:rows, 0:H1])

        stage_a(GROUPS_A, N_AGENTS, ALLY_F, fta_sb, W1A_OFF, ha_sb, alpha_a)
        stage_a(GROUPS_E, N_ENEMIES, ENEMY_F, fte_sb, W1E_OFF, he_sb, alpha_e)

        # ---- stage B (episode outer-product sums) ----
        def stage_b(groups, eg, n_per, featf, m_sb, h_sb, s_sb):
            gstride = eg * featf
            for gi, (e0, g) in enumerate(groups):
                rows = g * n_per
                width = g * featf
                moff = gi * gstride
                pb = psB.tile([H1, 512], F32, tag="psB")
                nc.tensor.matmul(
                    pb[:, 0:width],
                    h_sb[0:rows, gi * H1:(gi + 1) * H1],
                    m_sb[0:rows, moff:moff + width],
                    start=True, stop=True,
                )
                sfree = E_C * featf
                dst = _ap(s_sb, e0, [(sfree, H1), (1, g), (E_C, featf)])
                if gi % 2 == 0:
                    nc.vector.tensor_copy(dst, pb[:, 0:width])
                else:
                    nc.scalar.copy(dst, pb[:, 0:width])

        stage_b(GROUPS_A, EG_A, N_AGENTS, ALLY_F, ma_sb, ha_sb, sa_sb)
        stage_b(GROUPS_E, EG_E, N_ENEMIES, ENEMY_F, me_sb, he_sb, se_sb)

        # ---- stage C: out_T[o, e] accumulation over 80 f-slices ----
        pc = psC.tile([OUT, E_C], F32)
        n_slices = ALLY_F + ENEMY_F
        idx = 0
        for f in range(ALLY_F):
            nc.tensor.matmul(
                pc[:],
                w2_sb[:, W2A_OFF + f * OUT:W2A_OFF + (f + 1) * OUT],
                sa_sb[:, f * E_C:(f + 1) * E_C],
                start=(idx == 0), stop=(idx == n_slices - 1),
            )
            idx += 1
        for f in range(ENEMY_F):
            nc.tensor.matmul(
                pc[:],
                w2_sb[:, W2E_OFF + f * OUT:W2E_OFF + (f + 1) * OUT],
                se_sb[:, f * E_C:(f + 1) * E_C],
                start=(idx == 0), stop=(idx == n_slices - 1),
            )
            idx += 1

        nc.vector.tensor_copy(out_sb[:], pc[:])
        nc.sync.dma_start(out_d.ap(), out_sb[:])


@functools.lru_cache(maxsize=2)
def _cached_program(alpha_a, alpha_e):
    return build_program(alpha_a, alpha_e)


def host_prep(ally_features, enemy_features, Wa1, ba1, aa, Wa2, ba2,
              We1, be1, ae, We2, be2):
    """Build per-core input maps (numpy, bf16)."""
    bf = ml_dtypes.bfloat16

    def w1_pack(W1, b1, featf):
        w = np.zeros((H1, W1COLS), dtype=np.float32)
        w[0:featf, 0:HYPER] = np.asarray(W1)
        w[featf, 0:HYPER] = np.asarray(b1)
        w[featf, HYPER] = 1.0                 # ones column
        return w

    def uniform_alpha(a):
        a = np.asarray(a, dtype=np.float32)
        assert np.allclose(a, a[0]), "per-channel alpha not supported"
        assert 0.0 <= float(a[0]) <= 1.0, "alpha outside [0,1]"
        return float(a[0])

    ua, ue = uniform_alpha(aa), uniform_alpha(ae)
    w1a = w1_pack(Wa1, ba1, ALLY_F)
    w1e = w1_pack(We1, be1, ENEMY_F)

    w2 = np.zeros((H1, W2PACK_COLS), dtype=np.float32)
    w2[0:HYPER, W2A_OFF:W2A_OFF + W2A_LEN] = np.asarray(Wa2)
    w2[HYPER, W2A_OFF:W2A_OFF + W2A_LEN] = np.asarray(ba2)
    w2[0:HYPER, W2E_OFF:W2E_OFF + W2E_LEN] = np.asarray(We2)
    w2[HYPER, W2E_OFF:W2E_OFF + W2E_LEN] = np.asarray(be2)
    w2 = w2.astype(bf)
    w1 = np.zeros((128, W1PACK_COLS), dtype=np.float32)
    w1[0:H1, W1A_OFF:W1A_OFF + W1COLS] = w1a
    w1[0:H1, W1E_OFF:W1E_OFF + W1COLS] = w1e
    w1 = w1.astype(bf)

    fa_all = np.asarray(ally_features, dtype=np.float32).astype(bf)
    fe_all = np.asarray(enemy_features, dtype=np.float32).astype(bf)

    in_maps = []
    for c in range(N_CORES):
        fa_c = np.ascontiguousarray(fa_all[c * RA:(c + 1) * RA])
        fe_c = np.ascontiguousarray(fe_all[c * RE:(c + 1) * RE])
        fta_c = np.zeros((128, RA), dtype=np.float32)
        fta_c[0:ALLY_F] = fa_c.T.astype(np.float32)
        fta_c[ALLY_F] = 1.0
        fte_c = np.zeros((128, RE), dtype=np.float32)
        fte_c[0:ENEMY_F] = fe_c.T.astype(np.float32)
        fte_c[ENEMY_F] = 1.0
        in_maps.append({
            "fa": fa_c, "fe": fe_c,
            "fta": np.ascontiguousarray(fta_c.astype(bf)),
            "fte": np.ascontiguousarray(fte_c.astype(bf)),
            "w1pack": w1, "w2pack": w2,
        })
    return in_maps, ua, ue


def kernel(**inputs) -> np.ndarray:
    in_maps, ua, ue = host_prep(**inputs)
    nc = _cached_program(ua, ue)
    res = run_bass_kernel_spmd(nc, in_maps, core_ids=list(range(N_CORES)))
    outs = [np.asarray(r["out"], dtype=np.float32) for r in res.results]
    return np.concatenate([o.T for o in outs], axis=0)


if __name__ == "__main__":
    build_program()
    print("built ok")


# revision 31
# speedup vs baseline: 1.3839x; 1.1553x over previous
"""Trainium2 Bass kernel for nn_APIHyperInputLayer (hypernet MLP, 8-core data parallel).

Math (per branch):
    h   = prelu(F @ W1 + b1, alpha)                       [R, 64]
    w   = (h @ W2 + b2).reshape(R, F, 128)
    hid = einsum('rf,rfo->ro', F, w)
    out = hid.reshape(E, n, 128).sum(1)                   [E, 128]

Key restructuring: pull the agent-sum inside the W2 contraction.
    S[k, e, f]  = sum_i h[(e,i), k] * F[(e,i), f]     (outer-product episode sums)
    out[e, o]   = sum_{k,f} S[k, e, f] * W2[k, f*128+o]  (+ bias via Fsum row)
This cuts FLOPs ~8.7x vs materializing w.

On-chip schedule per core (256 episodes), all matmuls bf16 -> fp32 PSUM:
  A: h_aug[row, 0:65] = max(x, alpha*x) of F@[W1|b1-augmented]; col 64 == 1.
     (alpha is folded into a doubled W1; PReLU becomes one vector max)
  B: per group of 12 (ally) / 11 (enemy) episodes: S' = h_aug.T @ M where M is the
     block-diagonal masked feature tensor, built by one diagonal-AP DMA into
     pre-zeroed SBUF. S'[64] = per-episode feature sums (h_aug ones column).
  C: out_T[o, e] accumulated over 48+32 f-slices:
     out_T += W2aug[:, f*128:(f+1)*128].T @ S'[:, f::48]   (bias row folded).
Output per core: [128 o, 256 e] fp32; host transposes/concats.
"""

import os
import sys
import functools

import numpy as np

for _p in ("/opt/trn_rl_repo", os.path.expanduser("~/.axon_site/_ro/trn_rl_repo")):
    if os.path.isdir(_p) and _p not in sys.path:
        sys.path.insert(0, _p)

import dataclasses

import ml_dtypes

import concourse.bass as bass
import concourse.bacc as bacc
import concourse.mybir as mybir
import concourse.tile as tile
from concourse.bass_utils import run_bass_kernel_spmd

BF16 = mybir.dt.bfloat16
F32 = mybir.dt.float32

# Problem constants (hardcoded per contest rules)
N_CORES = 8
N_AGENTS, N_ENEMIES = 10, 11
ALLY_F, ENEMY_F = 48, 32
HYPER = 64
OUT = 128
B_FULL = 2048
E_C = B_FULL // N_CORES            # episodes per core = 256
RA = E_C * N_AGENTS                # ally rows per core = 2560
RE = E_C * N_ENEMIES               # enemy rows per core = 2816

# group sizes (episodes per stage-B matmul group; rows <= 128,
# group width * 4B <= 2KB so one stage-B matmul fits one PSUM bank)
EG_A = 10                          # 100 rows, width 480
EG_E = 11                          # 121 rows, width 352

H1 = HYPER + 1                     # 65: h columns + ones col
W1COLS = H1                        # 65: W1 cols + ones column

# w2pack column layout (bf16, 128 partitions; stage-C PAIRED weights:
# rows 0-63 = W2[:, f], rows 64-127 = W2[:, f + half] so each C matmul
# contracts 128 partitions (f-slice pairs); plus bias blocks)
PAIR_A = ALLY_F // 2               # 24 ally f-pairs
PAIR_E = ENEMY_F // 2              # 16 enemy f-pairs
W2A_OFF = 0
W2A_LEN = PAIR_A * OUT             # 3072
W2E_OFF = W2A_OFF + W2A_LEN
W2E_LEN = PAIR_E * OUT             # 2048
B2A_OFF = W2E_OFF + W2E_LEN
B2E_OFF = B2A_OFF + OUT
W2PACK_COLS = B2E_OFF + OUT
# w1pack: tiny, loaded first, padded to 128 partitions so every SDMA engine
# participates and the completion semaphore fires promptly
W1A_OFF = 0
W1E_OFF = W1COLS
W1PACK_COLS = 2 * W1COLS


def _groups(n_ep, eg):
    """List of (episode_start, n_episodes) per group."""
    out = []
    e = 0
    while e < n_ep:
        g = min(eg, n_ep - e)
        out.append((e, g))
        e += g
    return out


GROUPS_A = _groups(E_C, EG_A)      # 25 x 10 + 1 x 6
GROUPS_E = _groups(E_C, EG_E)      # 23 x 11 + 1 x 3
MA_FREE = len(GROUPS_A) * EG_A * ALLY_F     # M_all ally free size
ME_FREE = len(GROUPS_E) * EG_E * ENEMY_F
SA_FREE = E_C * ALLY_F             # 12288
SE_FREE = E_C * ENEMY_F            # 8192
HA_FREE = len(GROUPS_A) * H1
HE_FREE = len(GROUPS_E) * H1


def _ap(t, offset, dims):
    """Custom flat AP: dims = [(step, num), ...]; t is an AP or tensor handle."""
    a = t if isinstance(t, bass.AP) else t.ap()
    return dataclasses.replace(a, offset=offset, ap=[[s, n] for (s, n) in dims])


def build_program(alpha_a=0.25, alpha_e=0.25):
    nc = bacc.Bacc("TRN2", target_bir_lowering=False, debug=False)

    # DRAM parameters (per-core shards; bf16 except output)
    fa = nc.declare_dram_parameter("fa", [RA, ALLY_F], BF16, isOutput=False)
    fe = nc.declare_dram_parameter("fe", [RE, ENEMY_F], BF16, isOutput=False)
    fta = nc.declare_dram_parameter("fta", [128, RA], BF16, isOutput=False)
    fte = nc.declare_dram_parameter("fte", [128, RE], BF16, isOutput=False)
    w1pack = nc.declare_dram_parameter("w1pack", [128, W1PACK_COLS], BF16, isOutput=False)
    w2pack = nc.declare_dram_parameter("w2pack", [128, W2PACK_COLS], BF16, isOutput=False)
    out_d = nc.declare_dram_parameter("out", [OUT, E_C], F32, isOutput=True)

    with tile.TileContext(nc) as tc:
        _emit(nc, tc, fa, fe, fta, fte, w1pack, w2pack, out_d, alpha_a, alpha_e)
    nc.compile()
    return nc


def _emit(nc, tc, fa, fe, fta, fte, w1pack, w2pack, out_d, alpha_a=0.25, alpha_e=0.25):
    from contextlib import ExitStack

    ctx = ExitStack()
    with ctx:
        const = ctx.enter_context(tc.tile_pool(name="const", bufs=1))
        work = ctx.enter_context(tc.tile_pool(name="work", bufs=1))
        psA = ctx.enter_context(tc.tile_pool(name="psA", bufs=3, space="PSUM"))
        psB = ctx.enter_context(tc.tile_pool(name="psB", bufs=4, space="PSUM"))
        psC = ctx.enter_context(tc.tile_pool(name="psC", bufs=1, space="PSUM"))
        upool = ctx.enter_context(tc.tile_pool(name="upool", bufs=3))

        # ---- persistent SBUF buffers ----
        w1_sb = const.tile([128, W1PACK_COLS], BF16)
        w2_sb = const.tile([128, W2PACK_COLS], BF16)
        fta_sb = const.tile([128, RA], BF16)
        fte_sb = const.tile([128, RE], BF16)
        ma_sb = work.tile([128, MA_FREE], BF16)
        me_sb = work.tile([128, ME_FREE], BF16)
        ha_sb = work.tile([128, HA_FREE], BF16)
        he_sb = work.tile([128, HE_FREE], BF16)
        sa_sb = work.tile([128, SA_FREE], BF16)
        se_sb = work.tile([128, SE_FREE], BF16)
        fsum_sb = work.tile([128, 2 * E_C], BF16)
        out_sb = work.tile([OUT, E_C], F32)

        # ---- loads ----
        nc.sync.dma_start(w1_sb[:], w1pack.ap())
        nc.sync.dma_start(fta_sb[:], fta.ap())
        nc.scalar.dma_start(fte_sb[:], fte.ap())
        nc.scalar.dma_start(w2_sb[:], w2pack.ap())

        # zero the masked-feature buffers (ally first: its diag DMAs wait
        # DVE>=1 on fresh lanes; enemy ring-first DMAs get lanes 6,7 kept
        # fresh below so their DVE>=2 wait is their only one).
        nc.vector.memset(ma_sb[:].bitcast(F32), 0.0)
        nc.gpsimd.memset(me_sb[:].bitcast(F32), 0.0)

        # ---- diagonal DMAs: DRAM features -> block-diagonal M ----
        # One DMA per episode-slot e_local: for fixed e_local the SBUF
        # destination has pure strides (group dim steps free only, agent
        # dim steps whole partitions) so the BIR verifier accepts it.
        # HWDGE (sync/scalar) DMA instrs carry at most ONE sync wait;
        # SWDGE (gpsimd) waits are software -> flexible. Route the
        # dep-heavy first DMAs through gpsimd.
        dma_engines = [nc.scalar, nc.sync]
        dma_rr = [0]

        def diag_dma(m_sb, f_d, groups, eg, n_per, featf, mfree, swdge_els=()):
            gstride = eg * featf
            tail_g = groups[-1][1]          # episodes in last (ragged) group
            nfull = len(groups) - (1 if tail_g != eg else 0)
            for el in range(eg):
                ng = nfull + (1 if el < tail_g and tail_g != eg else 0)
                if el in swdge_els:
                    eng = nc.gpsimd
                else:
                    eng = dma_engines[dma_rr[0] % len(dma_engines)]
                    dma_rr[0] += 1
                eng.dma_start(
                    _ap(m_sb, (el * n_per) * mfree + el * featf, [
                        (mfree, n_per),         # agent: whole partitions
                        (gstride, ng),          # group: free step only
                        (1, featf),
                    ]),
                    _ap(f_d, el * n_per * featf, [
                        (featf, n_per),
                        (eg * n_per * featf, ng),
                        (1, featf),
                    ]),
                )

        diag_dma(ma_sb, fa, GROUPS_A, EG_A, N_AGENTS, ALLY_F, MA_FREE)
        diag_dma(me_sb, fe, GROUPS_E, EG_E, N_ENEMIES, ENEMY_F, ME_FREE)


        # ---- stage A (layer 1 + PReLU) for both branches ----
        # Two groups share one PSUM tile; PReLU (u = alpha*x; h = max(x,u))
        # is done once per pair to halve the DVE op count.
        def stage_a(groups, n_per, featf, ft_sb, w1_off, h_sb, alpha):
            fp1 = featf + 1
            pairs = [groups[i:i + 2] for i in range(0, len(groups), 2)]
            for pi, pair in enumerate(pairs):
                pa = psA.tile([128, 2 * W1COLS], F32, tag="psA")
                rows_l = []
                for slot, (e0, g) in enumerate(pair):
                    rows = g * n_per
                    rows_l.append(rows)
                    r0 = e0 * n_per
                    nc.tensor.matmul(
                        pa[0:rows, slot * W1COLS:(slot + 1) * W1COLS],
                        ft_sb[0:fp1, r0:r0 + rows],
                        w1_sb[0:fp1, w1_off:w1_off + W1COLS],
                        start=True, stop=True,
                    )
                gi0 = 2 * pi
                if len(pair) == 2 and rows_l[0] == rows_l[1]:
                    rows = rows_l[0]
                    ut = upool.tile([128, 2 * H1], BF16, tag="u")
                    src_ap = _ap(pa, 0, [(2 * W1COLS, rows), (W1COLS, 2), (1, H1)])
                    nc.vector.tensor_scalar_mul(ut[0:rows, :], src_ap, alpha)
                    nc.vector.tensor_max(
                        h_sb[0:rows, gi0 * H1:(gi0 + 2) * H1],
                        src_ap, ut[0:rows, :])
                else:
                    for slot in range(len(pair)):
                        rows = rows_l[slot]
                        ut = upool.tile([128, 2 * H1], BF16, tag="u")
                        nc.vector.tensor_scalar_mul(
                            ut[0:rows, 0:H1],
                            pa[0:rows, slot * W1COLS:slot * W1COLS + H1], alpha)
                        nc.vector.tensor_max(
                            h_sb[0:rows, (gi0 + slot) * H1:(gi0 + slot + 1) * H1],
                            pa[0:rows, slot * W1COLS:slot * W1COLS + H1],
                            ut[0:rows, 0:H1])

        stage_a(GROUPS_A, N_AGENTS, ALLY_F, fta_sb, W1A_OFF, ha_sb, alpha_a)
        stage_a(GROUPS_E, N_ENEMIES, ENEMY_F, fte_sb, W1E_OFF, he_sb, alpha_e)

        # ---- stage B (episode outer-product sums) ----
        def stage_b(groups, eg, n_per, featf, m_sb, h_sb, s_sb):
            gstride = eg * featf
            for gi, (e0, g) in enumerate(groups):
                rows = g * n_per
                width = g * featf
                moff = gi * gstride
                pb = psB.tile([H1, 512], F32, tag="psB")
                nc.tensor.matmul(
                    pb[:, 0:width],
                    h_sb[0:rows, gi * H1:(gi + 1) * H1],
                    m_sb[0:rows, moff:moff + width],
                    start=True, stop=True,
                )
                dst = s_sb[0:HYPER, e0 * featf:e0 * featf + width]
                if gi % 2 == 0:
                    nc.vector.tensor_copy(dst, pb[0:HYPER, 0:width])
                else:
                    nc.scalar.copy(dst, pb[0:HYPER, 0:width])

        stage_b(GROUPS_A, EG_A, N_AGENTS, ALLY_F, ma_sb, ha_sb, sa_sb)
        stage_b(GROUPS_E, EG_E, N_ENEMIES, ENEMY_F, me_sb, he_sb, se_sb)

        # Build the paired-S upper halves: rows 64-127 = rows 0-63 shifted
        # left by half-featf columns, so a stride-featf read at column
        # e*featf+f yields S[k,e,f] on rows 0-63 and S[k,e,f+half] above.
        def shift_dup(s_sb, featf, s_free, nchunk):
            half = (featf // 2)
            tot = s_free - half
            cs = (tot + nchunk - 1) // nchunk
            for i in range(nchunk):
                c0 = i * cs
                c1 = min(tot, c0 + cs)
                if c0 >= c1:
                    break
                eng = nc.sync if i % 2 == 0 else nc.scalar
                eng.dma_start(
                    _ap(s_sb, HYPER * s_free + c0, [(s_free, HYPER), (1, c1 - c0)]),
                    _ap(s_sb, half + c0, [(s_free, HYPER), (1, c1 - c0)]),
                )

        shift_dup(sa_sb, ALLY_F, SA_FREE, 4)
        shift_dup(se_sb, ENEMY_F, SE_FREE, 2)

        # Per-episode feature sums for the bias term: fsum[f, e]
        # (bf16 out is fine: sums of 10-11 unit-scale values)
        with nc.allow_low_precision(reason="bf16 episode feature sums"):
            nc.vector.reduce_sum(
                fsum_sb[0:ALLY_F, 0:E_C],
                _ap(fta_sb, 0, [(RA, ALLY_F), (N_AGENTS, E_C), (1, N_AGENTS)]),
                axis=mybir.AxisListType.X)
            nc.vector.reduce_sum(
                fsum_sb[0:ENEMY_F, E_C:2 * E_C],
                _ap(fte_sb, 0, [(RE, ENEMY_F), (N_ENEMIES, E_C), (1, N_ENEMIES)]),
                axis=mybir.AxisListType.X)

        # ---- stage C: out_T[o, e] accumulation over 80 f-slices ----
        pc = psC.tile([OUT, E_C], F32)
        n_slices = PAIR_A + PAIR_E + 2
        idx = 0
        for f in range(PAIR_A):
            nc.tensor.matmul(
                pc[:],
                w2_sb[:, W2A_OFF + f * OUT:W2A_OFF + (f + 1) * OUT],
                _ap(sa_sb, f, [(SA_FREE, 128), (ALLY_F, E_C)]),
                start=(idx == 0), stop=(idx == n_slices - 1),
            )
            idx += 1
        for f in range(PAIR_E):
            nc.tensor.matmul(
                pc[:],
                w2_sb[:, W2E_OFF + f * OUT:W2E_OFF + (f + 1) * OUT],
                _ap(se_sb, f, [(SE_FREE, 128), (ENEMY_F, E_C)]),
                start=(idx == 0), stop=(idx == n_slices - 1),
            )
            idx += 1
        nc.tensor.matmul(
            pc[:], w2_sb[0:ALLY_F, B2A_OFF:B2A_OFF + OUT],
            fsum_sb[0:ALLY_F, 0:E_C],
            start=False, stop=False)
        idx += 1
        nc.tensor.matmul(
            pc[:], w2_sb[0:ENEMY_F, B2E_OFF:B2E_OFF + OUT],
            fsum_sb[0:ENEMY_F, E_C:2 * E_C],
            start=False, stop=(idx == n_slices - 1))

        nc.vector.tensor_copy(out_sb[:], pc[:])
        nc.sync.dma_start(out_d.ap(), out_sb[:])


@functools.lru_cache(maxsize=2)
def _cached_program(alpha_a, alpha_e):
    return build_program(alpha_a, alpha_e)


def host_prep(ally_features, enemy_features, Wa1, ba1, aa, Wa2, ba2,
              We1, be1, ae, We2, be2):
    """Build per-core input maps (numpy, bf16)."""
    bf = ml_dtypes.bfloat16

    def w1_pack(W1, b1, featf):
        w = np.zeros((H1, W1COLS), dtype=np.float32)
        w[0:featf, 0:HYPER] = np.asarray(W1)
        w[featf, 0:HYPER] = np.asarray(b1)
        w[featf, HYPER] = 1.0                 # ones column
        return w

    def uniform_alpha(a):
        a = np.asarray(a, dtype=np.float32)
        assert np.allclose(a, a[0]), "per-channel alpha not supported"
        assert 0.0 <= float(a[0]) <= 1.0, "alpha outside [0,1]"
        return float(a[0])

    ua, ue = uniform_alpha(aa), uniform_alpha(ae)
    w1a = w1_pack(Wa1, ba1, ALLY_F)
    w1e = w1_pack(We1, be1, ENEMY_F)

    w2 = np.zeros((128, W2PACK_COLS), dtype=np.float32)
    Wa2_, We2_ = np.asarray(Wa2), np.asarray(We2)
    for f in range(PAIR_A):
        w2[0:HYPER, W2A_OFF + f * OUT:W2A_OFF + (f + 1) * OUT] = \
            Wa2_[:, f * OUT:(f + 1) * OUT]
        w2[HYPER - 1 + 65:] = w2[HYPER - 1 + 65:]  # noop keep shape
        w2[64:128, W2A_OFF + f * OUT:W2A_OFF + (f + 1) * OUT] = \
            Wa2_[:, (f + PAIR_A) * OUT:(f + PAIR_A + 1) * OUT]
    for f in range(PAIR_E):
        w2[0:HYPER, W2E_OFF + f * OUT:W2E_OFF + (f + 1) * OUT] = \
            We2_[:, f * OUT:(f + 1) * OUT]
        w2[64:128, W2E_OFF + f * OUT:W2E_OFF + (f + 1) * OUT] = \
            We2_[:, (f + PAIR_E) * OUT:(f + PAIR_E + 1) * OUT]
    w2[0:ALLY_F, B2A_OFF:B2A_OFF + OUT] = np.asarray(ba2).reshape(ALLY_F, OUT)
    w2[0:ENEMY_F, B2E_OFF:B2E_OFF + OUT] = np.asarray(be2).reshape(ENEMY_F, OUT)
    w2 = w2.astype(bf)
    w1 = np.zeros((128, W1PACK_COLS), dtype=np.float32)
    w1[0:H1, W1A_OFF:W1A_OFF + W1COLS] = w1a
    w1[0:H1, W1E_OFF:W1E_OFF + W1COLS] = w1e
    w1 = w1.astype(bf)

    fa_all = np.asarray(ally_features, dtype=np.float32).astype(bf)
    fe_all = np.asarray(enemy_features, dtype=np.float32).astype(bf)

    in_maps = []
    for c in range(N_CORES):
        fa_c = np.ascontiguousarray(fa_all[c * RA:(c + 1) * RA])
        fe_c = np.ascontiguousarray(fe_all[c * RE:(c + 1) * RE])
        fta_c = np.zeros((128, RA), dtype=np.float32)
        fta_c[0:ALLY_F] = fa_c.T.astype(np.float32)
        fta_c[ALLY_F] = 1.0
        fte_c = np.zeros((128, RE), dtype=np.float32)
        fte_c[0:ENEMY_F] = fe_c.T.astype(np.float32)
        fte_c[ENEMY_F] = 1.0
        in_maps.append({
            "fa": fa_c, "fe": fe_c,
            "fta": np.ascontiguousarray(fta_c.astype(bf)),
            "fte": np.ascontiguousarray(fte_c.astype(bf)),
            "w1pack": w1, "w2pack": w2,
        })
    return in_maps, ua, ue


def kernel(**inputs) -> np.ndarray:
    in_maps, ua, ue = host_prep(**inputs)
    nc = _cached_program(ua, ue)
    res = run_bass_kernel_spmd(nc, in_maps, core_ids=list(range(N_CORES)))
    outs = [np.asarray(r["out"], dtype=np.float32) for r in res.results]
    return np.concatenate([o.T for o in outs], axis=0)


if __name__ == "__main__":
    build_program()
    print("built ok")


# revision 33
# speedup vs baseline: 1.4686x; 1.0613x over previous
"""Trainium2 Bass kernel for nn_APIHyperInputLayer (hypernet MLP, 8-core data parallel).

Math (per branch):
    h   = prelu(F @ W1 + b1, alpha)                       [R, 64]
    w   = (h @ W2 + b2).reshape(R, F, 128)
    hid = einsum('rf,rfo->ro', F, w)
    out = hid.reshape(E, n, 128).sum(1)                   [E, 128]

Key restructuring: pull the agent-sum inside the W2 contraction.
    S[k, e, f]  = sum_i h[(e,i), k] * F[(e,i), f]     (outer-product episode sums)
    out[e, o]   = sum_{k,f} S[k, e, f] * W2[k, f*128+o]  (+ bias via Fsum row)
This cuts FLOPs ~8.7x vs materializing w.

On-chip schedule per core (256 episodes), all matmuls bf16 -> fp32 PSUM:
  A: x = F_aug @ [W1|b1] (ones row supplies the bias); PReLU via
     u = alpha*x then max(x, u), pair-batched on DVE.
  B: per group of 10 (ally) / 11 (enemy) episodes: S' = h_aug.T @ M where M
     is the block-diagonal masked feature tensor, built by per-episode-slot
     diagonal DMAs into pre-zeroed SBUF (one DMA per e_local keeps the APs
     partition-pure for the BIR verifier).
  C: PAIRED f-slices: S rows 64-127 = rows 0-63 shifted left by featf/2
     (two SBUF->SBUF DMAs), so each of the 24+16 accumulating matmuls
     contracts 128 partitions: out_T[o,e] += W2pair_f.T @ S_dup[:, f::featf].
     Bias via fsum[f,e] (DVE strided reduce of F^T) @ b2 reshaped.
Output per core: [128 o, 256 e] fp32; host transposes/concats.
HW-measured: 85.8us exec, rel err 3.8e-3 (vs 118.7us first working version).
"""

import os
import sys
import functools

import numpy as np

for _p in ("/opt/trn_rl_repo", os.path.expanduser("~/.axon_site/_ro/trn_rl_repo")):
    if os.path.isdir(_p) and _p not in sys.path:
        sys.path.insert(0, _p)

import dataclasses

import ml_dtypes

import concourse.bass as bass
import concourse.bacc as bacc
import concourse.mybir as mybir
import concourse.tile as tile
from concourse.bass_utils import run_bass_kernel_spmd

BF16 = mybir.dt.bfloat16
F32 = mybir.dt.float32

# Problem constants (hardcoded per contest rules)
N_CORES = 8
N_AGENTS, N_ENEMIES = 10, 11
ALLY_F, ENEMY_F = 48, 32
HYPER = 64
OUT = 128
B_FULL = 2048
E_C = B_FULL // N_CORES            # episodes per core = 256
RA = E_C * N_AGENTS                # ally rows per core = 2560
RE = E_C * N_ENEMIES               # enemy rows per core = 2816

# group sizes (episodes per stage-B matmul group; rows <= 128,
# group width * 4B <= 2KB so one stage-B matmul fits one PSUM bank)
EG_A = 10                          # 100 rows, width 480
EG_E = 11                          # 121 rows, width 352

H1 = HYPER + 1                     # 65: h columns + ones col
W1COLS = H1                        # 65: W1 cols + ones column

# w2pack column layout (bf16, 128 partitions; stage-C PAIRED weights:
# rows 0-63 = W2[:, f], rows 64-127 = W2[:, f + half] so each C matmul
# contracts 128 partitions (f-slice pairs); plus bias blocks)
PAIR_A = ALLY_F // 2               # 24 ally f-pairs
PAIR_E = ENEMY_F // 2              # 16 enemy f-pairs
W2A_OFF = 0
W2A_LEN = PAIR_A * OUT             # 3072
W2E_OFF = W2A_OFF + W2A_LEN
W2E_LEN = PAIR_E * OUT             # 2048
B2A_OFF = W2E_OFF + W2E_LEN
B2E_OFF = B2A_OFF + OUT
W2PACK_COLS = B2E_OFF + OUT
# w1pack: tiny, loaded first, padded to 128 partitions so every SDMA engine
# participates and the completion semaphore fires promptly
W1A_OFF = 0
W1E_OFF = W1COLS
W1PACK_COLS = 2 * W1COLS


def _groups(n_ep, eg):
    """List of (episode_start, n_episodes) per group."""
    out = []
    e = 0
    while e < n_ep:
        g = min(eg, n_ep - e)
        out.append((e, g))
        e += g
    return out


GROUPS_A = _groups(E_C, EG_A)      # 25 x 10 + 1 x 6
GROUPS_E = _groups(E_C, EG_E)      # 23 x 11 + 1 x 3
MA_FREE = len(GROUPS_A) * EG_A * ALLY_F     # M_all ally free size
ME_FREE = len(GROUPS_E) * EG_E * ENEMY_F
SA_FREE = E_C * ALLY_F             # 12288
SE_FREE = E_C * ENEMY_F            # 8192
HA_FREE = len(GROUPS_A) * H1
HE_FREE = len(GROUPS_E) * H1


def _ap(t, offset, dims):
    """Custom flat AP: dims = [(step, num), ...]; t is an AP or tensor handle."""
    a = t if isinstance(t, bass.AP) else t.ap()
    return dataclasses.replace(a, offset=offset, ap=[[s, n] for (s, n) in dims])


def build_program(alpha_a=0.25, alpha_e=0.25):
    nc = bacc.Bacc("TRN2", target_bir_lowering=False, debug=False)

    # DRAM parameters (per-core shards; bf16 except output)
    fa = nc.declare_dram_parameter("fa", [RA, ALLY_F], BF16, isOutput=False)
    fe = nc.declare_dram_parameter("fe", [RE, ENEMY_F], BF16, isOutput=False)
    fta = nc.declare_dram_parameter("fta", [ALLY_F + 1, RA], BF16, isOutput=False)
    fte = nc.declare_dram_parameter("fte", [ENEMY_F + 1, RE], BF16, isOutput=False)
    w1pack = nc.declare_dram_parameter("w1pack", [128, W1PACK_COLS], BF16, isOutput=False)
    w2pack = nc.declare_dram_parameter("w2pack", [128, W2PACK_COLS], BF16, isOutput=False)
    out_d = nc.declare_dram_parameter("out", [OUT, E_C], F32, isOutput=True)

    with tile.TileContext(nc) as tc:
        _emit(nc, tc, fa, fe, fta, fte, w1pack, w2pack, out_d, alpha_a, alpha_e)
    nc.compile()
    return nc


def _emit(nc, tc, fa, fe, fta, fte, w1pack, w2pack, out_d, alpha_a=0.25, alpha_e=0.25):
    from contextlib import ExitStack

    ctx = ExitStack()
    with ctx:
        const = ctx.enter_context(tc.tile_pool(name="const", bufs=1))
        work = ctx.enter_context(tc.tile_pool(name="work", bufs=1))
        psA = ctx.enter_context(tc.tile_pool(name="psA", bufs=3, space="PSUM"))
        psB = ctx.enter_context(tc.tile_pool(name="psB", bufs=4, space="PSUM"))
        psC = ctx.enter_context(tc.tile_pool(name="psC", bufs=1, space="PSUM"))
        upool = ctx.enter_context(tc.tile_pool(name="upool", bufs=3))

        # ---- persistent SBUF buffers ----
        w1_sb = const.tile([128, W1PACK_COLS], BF16)
        w2_sb = const.tile([128, W2PACK_COLS], BF16)
        fta_sb = const.tile([ALLY_F + 1, RA], BF16)
        fte_sb = const.tile([ENEMY_F + 1, RE], BF16)
        ma_sb = work.tile([128, MA_FREE], BF16)
        me_sb = work.tile([128, ME_FREE], BF16)
        ha_sb = work.tile([128, HA_FREE], BF16)
        he_sb = work.tile([128, HE_FREE], BF16)
        sa_sb = work.tile([128, SA_FREE], BF16)
        se_sb = work.tile([128, SE_FREE], BF16)
        fsum_sb = work.tile([128, 2 * E_C], BF16)
        out_sb = work.tile([OUT, E_C], F32)

        # ---- loads ----
        nc.sync.dma_start(w1_sb[:], w1pack.ap())
        nc.sync.dma_start(fta_sb[:], fta.ap())
        nc.scalar.dma_start(fte_sb[:], fte.ap())
        nc.scalar.dma_start(w2_sb[:], w2pack.ap())

        # zero the masked-feature buffers (ally first: its diag DMAs wait
        # DVE>=1 on fresh lanes; enemy ring-first DMAs get lanes 6,7 kept
        # fresh below so their DVE>=2 wait is their only one).
        ma_f32 = ma_sb[:].bitcast(F32)
        me_f32 = me_sb[:].bitcast(F32)
        ha_m = MA_FREE // 4   # f32 halves
        he_m = ME_FREE // 4
        nc.vector.memset(ma_f32[:, 0:ha_m], 0.0)
        nc.gpsimd.memset(ma_f32[:, ha_m:2 * ha_m], 0.0)
        nc.vector.memset(me_f32[:, 0:he_m], 0.0)
        nc.gpsimd.memset(me_f32[:, he_m:2 * he_m], 0.0)

        # ---- diagonal DMAs: DRAM features -> block-diagonal M ----
        # One DMA per episode-slot e_local: for fixed e_local the SBUF
        # destination has pure strides (group dim steps free only, agent
        # dim steps whole partitions) so the BIR verifier accepts it.
        # HWDGE (sync/scalar) DMA instrs carry at most ONE sync wait;
        # SWDGE (gpsimd) waits are software -> flexible. Route the
        # dep-heavy first DMAs through gpsimd.
        dma_engines = [nc.scalar, nc.sync]
        dma_rr = [0]

        def diag_dma(m_sb, f_d, groups, eg, n_per, featf, mfree, swdge_els=()):
            gstride = eg * featf
            tail_g = groups[-1][1]          # episodes in last (ragged) group
            nfull = len(groups) - (1 if tail_g != eg else 0)
            for el in range(eg):
                ng = nfull + (1 if el < tail_g and tail_g != eg else 0)
                if el in swdge_els:
                    eng = nc.gpsimd
                else:
                    eng = dma_engines[dma_rr[0] % len(dma_engines)]
                    dma_rr[0] += 1
                eng.dma_start(
                    _ap(m_sb, (el * n_per) * mfree + el * featf, [
                        (mfree, n_per),         # agent: whole partitions
                        (gstride, ng),          # group: free step only
                        (1, featf),
                    ]),
                    _ap(f_d, el * n_per * featf, [
                        (featf, n_per),
                        (eg * n_per * featf, ng),
                        (1, featf),
                    ]),
                )

        diag_dma(ma_sb, fa, GROUPS_A, EG_A, N_AGENTS, ALLY_F, MA_FREE)
        diag_dma(me_sb, fe, GROUPS_E, EG_E, N_ENEMIES, ENEMY_F, ME_FREE)


        # ---- stage A (layer 1 + PReLU) for both branches ----
        # Two groups share one PSUM tile; PReLU (u = alpha*x; h = max(x,u))
        # is done once per pair to halve the DVE op count.
        def stage_a(groups, n_per, featf, ft_sb, w1_off, h_sb, alpha):
            fp1 = featf + 1
            pairs = [groups[i:i + 2] for i in range(0, len(groups), 2)]
            for pi, pair in enumerate(pairs):
                pa = psA.tile([128, 2 * W1COLS], F32, tag="psA")
                rows_l = []
                for slot, (e0, g) in enumerate(pair):
                    rows = g * n_per
                    rows_l.append(rows)
                    r0 = e0 * n_per
                    nc.tensor.matmul(
                        pa[0:rows, slot * W1COLS:(slot + 1) * W1COLS],
                        ft_sb[0:fp1, r0:r0 + rows],
                        w1_sb[0:fp1, w1_off:w1_off + W1COLS],
                        start=True, stop=True,
                    )
                gi0 = 2 * pi
                if len(pair) == 2 and rows_l[0] == rows_l[1]:
                    rows = rows_l[0]
                    ut = upool.tile([128, 2 * H1], BF16, tag="u")
                    src_ap = _ap(pa, 0, [(2 * W1COLS, rows), (W1COLS, 2), (1, H1)])
                    nc.vector.tensor_scalar_mul(ut[0:rows, :], src_ap, alpha)
                    nc.vector.tensor_max(
                        h_sb[0:rows, gi0 * H1:(gi0 + 2) * H1],
                        src_ap, ut[0:rows, :])
                else:
                    for slot in range(len(pair)):
                        rows = rows_l[slot]
                        ut = upool.tile([128, 2 * H1], BF16, tag="u")
                        nc.vector.tensor_scalar_mul(
                            ut[0:rows, 0:H1],
                            pa[0:rows, slot * W1COLS:slot * W1COLS + H1], alpha)
                        nc.vector.tensor_max(
                            h_sb[0:rows, (gi0 + slot) * H1:(gi0 + slot + 1) * H1],
                            pa[0:rows, slot * W1COLS:slot * W1COLS + H1],
                            ut[0:rows, 0:H1])

        stage_a(GROUPS_A, N_AGENTS, ALLY_F, fta_sb, W1A_OFF, ha_sb, alpha_a)
        stage_a(GROUPS_E, N_ENEMIES, ENEMY_F, fte_sb, W1E_OFF, he_sb, alpha_e)

        # ---- stage B (episode outer-product sums) ----
        def stage_b(groups, eg, n_per, featf, m_sb, h_sb, s_sb):
            gstride = eg * featf
            for gi, (e0, g) in enumerate(groups):
                rows = g * n_per
                width = g * featf
                moff = gi * gstride
                pb = psB.tile([H1, 512], F32, tag="psB")
                nc.tensor.matmul(
                    pb[:, 0:width],
                    h_sb[0:rows, gi * H1:(gi + 1) * H1],
                    m_sb[0:rows, moff:moff + width],
                    start=True, stop=True,
                )
                dst = s_sb[0:HYPER, e0 * featf:e0 * featf + width]
                if gi % 2 == 0:
                    nc.vector.tensor_copy(dst, pb[0:HYPER, 0:width])
                else:
                    nc.scalar.copy(dst, pb[0:HYPER, 0:width])

        stage_b(GROUPS_A, EG_A, N_AGENTS, ALLY_F, ma_sb, ha_sb, sa_sb)
        stage_b(GROUPS_E, EG_E, N_ENEMIES, ENEMY_F, me_sb, he_sb, se_sb)

        # Build the paired-S upper halves: rows 64-127 = rows 0-63 shifted
        # left by half-featf columns, so a stride-featf read at column
        # e*featf+f yields S[k,e,f] on rows 0-63 and S[k,e,f+half] above.
        def shift_dup(s_sb, featf, s_free, nchunk):
            half = (featf // 2)
            tot = s_free - half
            cs = (tot + nchunk - 1) // nchunk
            for i in range(nchunk):
                c0 = i * cs
                c1 = min(tot, c0 + cs)
                if c0 >= c1:
                    break
                eng = nc.sync if i % 2 == 0 else nc.scalar
                eng.dma_start(
                    _ap(s_sb, HYPER * s_free + c0, [(s_free, HYPER), (1, c1 - c0)]),
                    _ap(s_sb, half + c0, [(s_free, HYPER), (1, c1 - c0)]),
                )

        shift_dup(sa_sb, ALLY_F, SA_FREE, 4)
        shift_dup(se_sb, ENEMY_F, SE_FREE, 2)

        # Per-episode feature sums for the bias term: fsum[f, e]
        # (bf16 out is fine: sums of 10-11 unit-scale values)
        with nc.allow_low_precision(reason="bf16 episode feature sums"):
            nc.vector.reduce_sum(
                fsum_sb[0:ALLY_F, 0:E_C],
                _ap(fta_sb, 0, [(RA, ALLY_F), (N_AGENTS, E_C), (1, N_AGENTS)]),
                axis=mybir.AxisListType.X)
            nc.vector.reduce_sum(
                fsum_sb[0:ENEMY_F, E_C:2 * E_C],
                _ap(fte_sb, 0, [(RE, ENEMY_F), (N_ENEMIES, E_C), (1, N_ENEMIES)]),
                axis=mybir.AxisListType.X)

        # ---- stage C: out_T[o, e] accumulation over 80 f-slices ----
        pc = psC.tile([OUT, E_C], F32)
        n_slices = PAIR_A + PAIR_E + 2
        idx = 0
        for f in range(PAIR_A):
            nc.tensor.matmul(
                pc[:],
                w2_sb[:, W2A_OFF + f * OUT:W2A_OFF + (f + 1) * OUT],
                _ap(sa_sb, f, [(SA_FREE, 128), (ALLY_F, E_C)]),
                start=(idx == 0), stop=(idx == n_slices - 1),
            )
            idx += 1
        for f in range(PAIR_E):
            nc.tensor.matmul(
                pc[:],
                w2_sb[:, W2E_OFF + f * OUT:W2E_OFF + (f + 1) * OUT],
                _ap(se_sb, f, [(SE_FREE, 128), (ENEMY_F, E_C)]),
                start=(idx == 0), stop=(idx == n_slices - 1),
            )
            idx += 1
        nc.tensor.matmul(
            pc[:], w2_sb[0:ALLY_F, B2A_OFF:B2A_OFF + OUT],
            fsum_sb[0:ALLY_F, 0:E_C],
            start=False, stop=False)
        idx += 1
        nc.tensor.matmul(
            pc[:], w2_sb[0:ENEMY_F, B2E_OFF:B2E_OFF + OUT],
            fsum_sb[0:ENEMY_F, E_C:2 * E_C],
            start=False, stop=(idx == n_slices - 1))

        nc.vector.tensor_copy(out_sb[:], pc[:])
        nc.sync.dma_start(out_d.ap(), out_sb[:])


@functools.lru_cache(maxsize=2)
def _cached_program(alpha_a, alpha_e):
    return build_program(alpha_a, alpha_e)


def host_prep(ally_features, enemy_features, Wa1, ba1, aa, Wa2, ba2,
              We1, be1, ae, We2, be2):
    """Build per-core input maps (numpy, bf16)."""
    bf = ml_dtypes.bfloat16

    def w1_pack(W1, b1, featf):
        w = np.zeros((H1, W1COLS), dtype=np.float32)
        w[0:featf, 0:HYPER] = np.asarray(W1)
        w[featf, 0:HYPER] = np.asarray(b1)
        w[featf, HYPER] = 1.0                 # ones column
        return w

    def uniform_alpha(a):
        a = np.asarray(a, dtype=np.float32)
        assert np.allclose(a, a[0]), "per-channel alpha not supported"
        assert 0.0 <= float(a[0]) <= 1.0, "alpha outside [0,1]"
        return float(a[0])

    ua, ue = uniform_alpha(aa), uniform_alpha(ae)
    w1a = w1_pack(Wa1, ba1, ALLY_F)
    w1e = w1_pack(We1, be1, ENEMY_F)

    w2 = np.zeros((128, W2PACK_COLS), dtype=np.float32)
    Wa2_, We2_ = np.asarray(Wa2), np.asarray(We2)
    for f in range(PAIR_A):
        w2[0:HYPER, W2A_OFF + f * OUT:W2A_OFF + (f + 1) * OUT] = \
            Wa2_[:, f * OUT:(f + 1) * OUT]
        w2[HYPER - 1 + 65:] = w2[HYPER - 1 + 65:]  # noop keep shape
        w2[64:128, W2A_OFF + f * OUT:W2A_OFF + (f + 1) * OUT] = \
            Wa2_[:, (f + PAIR_A) * OUT:(f + PAIR_A + 1) * OUT]
    for f in range(PAIR_E):
        w2[0:HYPER, W2E_OFF + f * OUT:W2E_OFF + (f + 1) * OUT] = \
            We2_[:, f * OUT:(f + 1) * OUT]
        w2[64:128, W2E_OFF + f * OUT:W2E_OFF + (f + 1) * OUT] = \
            We2_[:, (f + PAIR_E) * OUT:(f + PAIR_E + 1) * OUT]
    w2[0:ALLY_F, B2A_OFF:B2A_OFF + OUT] = np.asarray(ba2).reshape(ALLY_F, OUT)
    w2[0:ENEMY_F, B2E_OFF:B2E_OFF + OUT] = np.asarray(be2).reshape(ENEMY_F, OUT)
    w2 = w2.astype(bf)
    w1 = np.zeros((128, W1PACK_COLS), dtype=np.float32)
    w1[0:H1, W1A_OFF:W1A_OFF + W1COLS] = w1a
    w1[0:H1, W1E_OFF:W1E_OFF + W1COLS] = w1e
    w1 = w1.astype(bf)

    fa_all = np.asarray(ally_features, dtype=np.float32).astype(bf)
    fe_all = np.asarray(enemy_features, dtype=np.float32).astype(bf)

    in_maps = []
    for c in range(N_CORES):
        fa_c = np.ascontiguousarray(fa_all[c * RA:(c + 1) * RA])
        fe_c = np.ascontiguousarray(fe_all[c * RE:(c + 1) * RE])
        fta_c = np.zeros((ALLY_F + 1, RA), dtype=np.float32)
        fta_c[0:ALLY_F] = fa_c.T.astype(np.float32)
        fta_c[ALLY_F] = 1.0
        fte_c = np.zeros((ENEMY_F + 1, RE), dtype=np.float32)
        fte_c[0:ENEMY_F] = fe_c.T.astype(np.float32)
        fte_c[ENEMY_F] = 1.0
        in_maps.append({
            "fa": fa_c, "fe": fe_c,
            "fta": np.ascontiguousarray(fta_c.astype(bf)),
            "fte": np.ascontiguousarray(fte_c.astype(bf)),
            "w1pack": w1, "w2pack": w2,
        })
    return in_maps, ua, ue


def kernel(**inputs) -> np.ndarray:
    in_maps, ua, ue = host_prep(**inputs)
    nc = _cached_program(ua, ue)
    res = run_bass_kernel_spmd(nc, in_maps, core_ids=list(range(N_CORES)))
    outs = [np.asarray(r["out"], dtype=np.float32) for r in res.results]
    return np.concatenate([o.T for o in outs], axis=0)


if __name__ == "__main__":
    build_program()
    print("built ok")
